# revision 41
# baseline (speedup 1.0000x reference)
"""Causal self-attention (B=4, T=2048, E=1024, H=16) on 8 trn2 NeuronCores.

Sharding: core c -> (batch b = c // 2, head-group hg = c % 2); each core owns
one batch element and 8 of the 16 heads (data parallel on B, tensor parallel
on heads).  No cross-core communication.

Per-core device program (SPMD, same NEFF on all 8 cores).  All 160 attention
items (I-block, head-pair, j-tile) run as ONE stream; the QKV projection
groups for block tb+1 are woven into the stream by a greedy scheduler that
keeps the PE fed without starving ScalarE (est-cost balance + hard deadlines
before each consuming unit).

  x / w in bf16 (same PE cost as fp32r, half the DMA bytes).
  q/k bias adds on DVE (tensor_scalar_add), v bias via K=1 ones matmul;
    ScalarE runs exp only.
  QK: row-tile pair, 2 heads per [128,1024] PSUM st tile (3 bufs).
  PV: full-128 contraction per head into one [65,1024] PSUM yt (row 64 =
    softmax denominator via a ones column in v).
  Causal: j-tiles past the diagonal are skipped; on-diagonal tiles are
    width-trimmed (QK moving, exp, PV moving) and the 128-wide boundary
    tile masked with a gpsimd affine_select after exp.
  Output: yt -> SBUF copy, reciprocal of row 64, gpsimd partition_broadcast,
    two DVE muls, DMA out of the DVE queue (keeps SP free for loads).
"""

import sys

sys.path.insert(0, "/opt/trn_rl_repo")

import numpy as np

N_CORES = 8
B, T, E = 4, 2048, 1024
H, D = 16, 64
C = E                 # q/k/v channel count (4th qkv chunk unused)
HPC = H // 2          # heads per core
CC = HPC * D          # per-core channels = 512
ES = E // 128         # 8 e-tiles (contraction)
TB = T // 512         # 4 t/i blocks of 512
NJ = T // 128         # 16 j-tiles of 128
PAIRS = HPC // 2      # 4 head pairs per core

CCV = HPC * (D + 1)   # v channels incl. a ones column per head = 520

ST_BUFS = 2          # attention QK->exp PSUM tiles (2 banks each)
GP_BUFS = 2          # projection-group PSUM tiles (1 bank each)
PT_BUFS = 8
LOOKAHEAD = 3

EMIT_LOG = {"ACT": [], "PE": []}  # emission-order labels, for trace analysis

_cache = {}


def _build_nc():
    import concourse.mybir as mybir
    import concourse.tile as tile
    from concourse import bacc

    f32 = mybir.dt.float32
    f32r = mybir.dt.float32r
    bf16 = mybir.dt.bfloat16
    Act = mybir.ActivationFunctionType
    is_ge = mybir.AluOpType.is_ge

    nc = bacc.Bacc("TRN2", target_bir_lowering=False, debug=False)

    xT = nc.dram_tensor("xT", [E, T], bf16, kind="ExternalInput").ap()
    w_qk = nc.dram_tensor("w_qk", [E, 2 * CC], bf16, kind="ExternalInput").ap()
    w_v = nc.dram_tensor("w_v", [E, CC], bf16, kind="ExternalInput").ap()
    b_qk = nc.dram_tensor("b_qk", [128, 8], f32, kind="ExternalInput").ap()
    b_v = nc.dram_tensor("b_v", [1, CC], f32r, kind="ExternalInput").ap()
    ones_d = nc.dram_tensor("ones_d", [1, 128], f32r, kind="ExternalInput").ap()
    yT = nc.dram_tensor("yT", [CC, T], f32, kind="ExternalOutput").ap()

    with tile.TileContext(nc) as tc:
        with (
            tc.tile_pool(name="persist", bufs=1) as pp,
            tc.tile_pool(name="psum", bufs=1, space="PSUM") as psp,
            tc.tile_pool(name="xpool", bufs=2) as xp,
            tc.tile_pool(name="qpool", bufs=2) as qp,
            tc.tile_pool(name="ptpool", bufs=1) as ptp,
            tc.tile_pool(name="opool", bufs=1) as op,
        ):
            # ---- persistent SBUF state ----
            k_sb = [pp.tile([128, T], f32r, name=f"k{p}") for p in range(PAIRS)]
            # v plus a ones column per head: [t-part, head, t-tile, 65]
            v1_sb = pp.tile([128, HPC, NJ, D + 1], f32r, name="v1")
            bqk_sb = pp.tile([128, 8], f32, name="bqk")
            bv_sb = pp.tile([1, CC], f32r, name="bv")
            ones_sb = pp.tile([1, 128], f32r, name="ones")
            wqk_h = [pp.tile([128, ES * 512], bf16, name=f"wqk{h}") for h in range(2)]
            wv_all = pp.tile([128, ES * 512], bf16, name="wv")

            # softmax-denominator ones column of v1: memset rejects f32r, so
            # stage a broadcast ones tile and strided-copy it in on DVE
            ones128 = pp.tile([128, 128], f32r, name="ones128")

            xs_tb = {}

            def load_x(tb):
                xt = xp.tile([128, ES * 512], bf16, tag="x", bufs=3,
                             name=f"x{tb}")
                nc.sync.dma_start(
                    out=xt,
                    in_=_mk_ap(xT, tb * 512, [[T, 128], [128 * T, ES], [1, 512]]),
                )
                xs_tb[tb] = xt

            # DMA order: x(0) + pair-0 q/k weights first (attention starts
            # earliest), then consts, v weights, pair-1 weights, x(1), rest
            x0 = xp.tile([128, ES * 512], bf16, tag="x", bufs=3, name="x0")
            xs_tb[0] = x0
            x0_e = x0.rearrange("p (e c) -> p e c", e=ES)
            nc.sync.dma_start(
                out=x0_e[:, 0:2, :],
                in_=_mk_ap(xT, 0, [[T, 128], [128 * T, 2], [1, 512]]),
            )
            wqk0_e = wqk_h[0].rearrange("p (e c) -> p e c", e=ES)
            nc.sync.dma_start(
                out=wqk0_e[:, 0:2, 0:256],
                in_=_mk_ap(w_qk, 0, [[1024, 128], [128 * 1024, 2], [1, 256]]),
            )
            nc.sync.dma_start(
                out=x0_e[:, 2:4, :],
                in_=_mk_ap(xT, 2 * 128 * T, [[T, 128], [128 * T, 2], [1, 512]]),
            )
            nc.sync.dma_start(
                out=wqk0_e[:, 2:ES, 0:256],
                in_=_mk_ap(w_qk, 2 * 128 * 1024,
                           [[1024, 128], [128 * 1024, ES - 2], [1, 256]]),
            )
            nc.sync.dma_start(out=bqk_sb, in_=b_qk)
            nc.sync.dma_start(
                out=x0_e[:, 4:ES, :],
                in_=_mk_ap(xT, 4 * 128 * T,
                           [[T, 128], [128 * T, ES - 4], [1, 512]]),
            )
            nc.sync.dma_start(out=bv_sb, in_=b_v)
            nc.sync.dma_start(out=ones_sb, in_=ones_d)
            nc.sync.dma_start(out=ones128, in_=_bcast_ap(ones_d, 128))
            nc.vector.tensor_copy(v1_sb[:, :, :, D : D + 1], ones128)
            nc.sync.dma_start(
                out=wv_all,
                in_=_mk_ap(w_v, 0, [[512, 128], [128 * 512, ES], [1, 512]]),
            )
            nc.sync.dma_start(
                out=wqk0_e[:, :, 256:512],
                in_=_mk_ap(w_qk, 256,
                           [[1024, 128], [128 * 1024, ES], [1, 256]]),
            )
            load_x(1)
            nc.sync.dma_start(
                out=wqk_h[1],
                in_=_mk_ap(w_qk, 512, [[1024, 128], [128 * 1024, ES], [1, 512]]),
            )

            # ---- QKV projection groups ----
            q_tiles = {}

            pend_g = {}

            def g_qk_half(tb, g, half):
                p, is_k = divmod(g, 2)
                h, sub = divmod(p, 2)
                xs = xs_tb[tb]
                es = range(ES // 2) if half == 0 else range(ES // 2, ES)
                EMIT_LOG["PE"] += [f"gqk{tb}_{g}.e{e}" for e in es]
                if half == 0:
                    ps = psp.tile([128, 512], f32, tag="gp", bufs=GP_BUFS,
                                  name=f"psqk{g}_{tb}")
                    pend_g[(tb, g)] = ps
                else:
                    ps = pend_g.pop((tb, g))
                for e in es:
                    c0 = e * 512 + 256 * sub + 128 * is_k
                    nc.tensor.matmul(
                        ps, wqk_h[h][:, c0 : c0 + 128],
                        xs[:, e * 512 : (e + 1) * 512],
                        start=(e == 0), stop=(e == ES - 1),
                    )
                if half == 1:
                    if is_k:
                        dest = k_sb[p][:, tb * 512 : (tb + 1) * 512]
                    else:
                        qt = qp.tile([128, 512], f32r, tag=f"q{p}", bufs=2,
                                     name=f"q{p}_{tb}")
                        q_tiles[(p, tb)] = qt
                        dest = qt
                    nc.vector.tensor_scalar_add(dest, ps, bqk_sb[:, g : g + 1])

            def g_qk(tb, g):
                g_qk_half(tb, g, 0)
                g_qk_half(tb, g, 1)

            def g_v_half(tb, k4, half):
                xs = xs_tb[tb]
                tt = tb * 4 + k4
                es = range(ES // 2) if half == 0 else range(ES // 2, ES)
                if half == 0:
                    EMIT_LOG["PE"] += [f"gv{tb}_{k4}.b"]
                EMIT_LOG["PE"] += [f"gv{tb}_{k4}.e{e}" for e in es]
                if half == 0:
                    psv = psp.tile([128, 512], f32, tag="gp", bufs=GP_BUFS,
                                   name=f"psv{tt}")
                    pend_g[("v", tt)] = psv
                    nc.tensor.matmul(
                        psv, ones_sb, bv_sb,
                        start=True, stop=False, skip_group_check=True,
                    )
                else:
                    psv = pend_g.pop(("v", tt))
                for e in es:
                    nc.tensor.matmul(
                        psv,
                        xs[:, e * 512 + 128 * k4 : e * 512 + 128 * (k4 + 1)],
                        wv_all[:, e * 512 : (e + 1) * 512],
                        start=False, stop=(e == ES - 1),
                        skip_group_check=True,
                    )
                if half == 1:
                    nc.vector.tensor_copy(
                        v1_sb[:, :, tt, 0:D],
                        psv.rearrange("p (h d) -> p h d", d=D),
                    )

            def g_v(tb, k4):
                g_v_half(tb, k4, 0)
                g_v_half(tb, k4, 1)

            # ---- attention ----
            pts = {}
            yts = {}

            def qk_exp(I, pr, J):
                EMIT_LOG["PE"] += [f"qk{I}{pr}{J}.A", f"qk{I}{pr}{J}.B"]
                r = J - 4 * I
                EMIT_LOG["ACT"] += [f"exp{I}{pr}{J}"]
                w0 = 128 * r if r >= 1 else 0
                qt = q_tiles[(pr, I)]
                kt = k_sb[pr]
                jsl = slice(J * 128, (J + 1) * 128)
                st = psp.tile([128, 1024], f32, tag="st", bufs=ST_BUFS,
                              name=f"st{pr}_{I}_{J}")
                nc.tensor.matmul(
                    st[:, w0:512], kt[0:64, jsl], qt[0:64, w0:512],
                    tile_position=(0, 0),
                )
                nc.tensor.matmul(
                    st[:, 512 + w0 : 1024], kt[64:128, jsl], qt[64:128, w0:512],
                    tile_position=(64, 0),
                )
                pt = ptp.tile([128, 1024], f32r, tag="pt", bufs=PT_BUFS,
                              name=f"pt{pr}_{I}_{J}")
                Exp = Act.Exp
                if r < 1:
                    nc.scalar.activation(pt, st, Exp, scale=0.125)
                else:  # one activation over both heads' trimmed windows
                    pt2 = pt.rearrange("p (h w) -> p h w", h=2)
                    st2 = st.rearrange("p (h w) -> p h w", h=2)
                    nc.scalar.activation(pt2[:, :, w0:512], st2[:, :, w0:512],
                                         Exp, scale=0.125)
                if r >= 0:  # mask the 128-wide boundary tile: keep col >= row
                    for off in (0, 512):
                        nc.gpsimd.affine_select(
                            out=pt[:, off + w0 : off + w0 + 128],
                            in_=pt[:, off + w0 : off + w0 + 128],
                            compare_op=is_ge,
                            fill=0.0,
                            base=0,
                            pattern=[[1, 128]],
                            channel_multiplier=-1,
                        )
                pts[(I, pr, J)] = pt

            def pv(I, pr, J):
                EMIT_LOG["PE"] += [f"pv{I}{pr}{J}.A", f"pv{I}{pr}{J}.B"]
                pt = pts.pop((I, pr, J))
                r = J - 4 * I
                w0 = 128 * r if r >= 1 else 0
                first, last = (J == 0), (J == 4 * I + 3)
                if first:
                    yts[pr] = (
                        psp.tile([D + 1, 512], f32, tag="ytA", bufs=1,
                                 name=f"ytA{pr}_{I}"),
                        psp.tile([D + 1, 512], f32, tag="ytB", bufs=1,
                                 name=f"ytB{pr}_{I}"),
                    )
                ytA, ytB = yts[pr]
                nc.tensor.matmul(
                    ytA[:, w0:512], v1_sb[:, 2 * pr, J, :], pt[:, w0:512],
                    start=first, stop=last, skip_group_check=True,
                )
                nc.tensor.matmul(
                    ytB[:, w0:512], v1_sb[:, 2 * pr + 1, J, :],
                    pt[:, 512 + w0 : 1024],
                    start=first, stop=last, skip_group_check=True,
                )

            def out_stage(I, pr, use_act=True, use_pe_bcast=False,
                          muls_on_pool=False):
                ytA, ytB = yts.pop(pr)
                tmpA = op.tile([D + 1, 512], f32, tag="tmpA", bufs=2,
                               name=f"tmpA{pr}_{I}")
                tmpB = op.tile([D + 1, 512], f32, tag="tmpB", bufs=2,
                               name=f"tmpB{pr}_{I}")
                nc.vector.tensor_copy(tmpA, ytA)   # frees head-A psum bank
                if use_act:  # ScalarE idles early in the run, DVE late
                    nc.scalar.copy(tmpB, ytB)
                    EMIT_LOG["ACT"].append(f"ycopy{pr}_{I}")
                else:
                    nc.vector.tensor_copy(tmpB, ytB)
                rec = op.tile([1, 1024], f32r, tag="rec", bufs=2,
                              name=f"rec{pr}_{I}")
                with nc.allow_low_precision(reason="f32r is bit-identical f32"):
                    nc.vector.reciprocal(rec[:, 0:512], tmpA[D : D + 1, :])
                    nc.vector.reciprocal(rec[:, 512:1024],
                                         tmpB[D : D + 1, :])
                ystage = op.tile([128, 512], f32, tag="ystage", bufs=2,
                                 name=f"ys{pr}_{I}")
                if use_pe_bcast:
                    # ACT-bound tail: PE has slack, broadcast 1/sum as
                    # ones^T @ rec into two 1-bank group-tag psum tiles
                    rbpA = psp.tile([64, 512], f32, tag="gp", bufs=GP_BUFS,
                                    name=f"rbpA{pr}_{I}")
                    rbpB = psp.tile([64, 512], f32, tag="gp", bufs=GP_BUFS,
                                    name=f"rbpB{pr}_{I}")
                    EMIT_LOG["PE"] += [f"rbc{pr}_{I}.A", f"rbc{pr}_{I}.B"]
                    with nc.allow_low_precision(reason="K=1 broadcast copy"):
                        nc.tensor.matmul(rbpA, ones_sb[:, 0:64], rec[:, 0:512])
                        nc.tensor.matmul(rbpB, ones_sb[:, 0:64],
                                         rec[:, 512:1024])
                    rbcA, rbcB = rbpA, rbpB
                else:
                    rbc = op.tile([64, 1024], f32r, tag="rbc", bufs=2,
                                  name=f"rbc{pr}_{I}")
                    nc.gpsimd.partition_broadcast(rbc, rec)
                    rbcA, rbcB = rbc[:, 0:512], rbc[:, 512:1024]
                mul1 = nc.gpsimd.tensor_mul if muls_on_pool else nc.vector.tensor_mul
                if muls_on_pool and not use_pe_bcast:
                    nc.gpsimd.tensor_mul(ystage[0:64, :], tmpA[0:D, :], rbcA)
                    nc.vector.tensor_mul(ystage[64:128, :], tmpB[0:D, :], rbcB)
                else:
                    nc.vector.tensor_mul(ystage[0:64, :], tmpA[0:D, :], rbcA)
                    nc.vector.tensor_mul(ystage[64:128, :], tmpB[0:D, :], rbcB)
                nc.sync.dma_start(
                    out=yT[pr * 128 : (pr + 1) * 128, I * 512 : (I + 1) * 512],
                    in_=ystage)

            # ---- schedule ----
            # Unit = (I, pr): all causal j-tiles of one head-pair/query-block.
            # Units run in an order that defers each block's last pairs to the
            # end, so projection groups (PE-only work) remain available to
            # fill the ACT-bound final stretch.  Each unit's weave list (the
            # groups feeding upcoming units) is emitted one group per item --
            # bursts of group psums stall the QK pipeline behind their
            # bias-add release.
            order = [(0, 0), (0, 1), (0, 2), (0, 3),
                     (1, 0), (1, 1), (1, 2),
                     (2, 0), (2, 1), (2, 2),
                     (3, 0), (3, 1),
                     (1, 3), (2, 3), (3, 2), (3, 3)]
            upos = {u: i for i, u in enumerate(order)}
            items = [(I, p, J) for (I, p) in order for J in range(4 * (I + 1))]

            def G(tb, g, half):
                return lambda: g_qk_half(tb, g, half)

            def V(tb, k4, half):
                return lambda: g_v_half(tb, k4, half)

            def GH(tb, g):
                return [G(tb, g, 0), G(tb, g, 1)]

            def VH(tb, k4):
                return [V(tb, k4, 0), V(tb, k4, 1)]

            wl = {i: [] for i in range(len(order))}
            wl[0] = GH(0, 2) + GH(0, 3)
            wl[1] = GH(0, 4) + GH(0, 5)
            wl[2] = GH(0, 6) + GH(0, 7) + GH(1, 0)
            wl[3] = GH(1, 1) + VH(1, 0) + VH(1, 1)
            wl[4] = VH(1, 2) + VH(1, 3) + GH(1, 2) + GH(1, 3)
            wl[5] = GH(1, 4) + GH(1, 5) + GH(2, 0)
            wl[6] = GH(2, 1) + VH(2, 0) + VH(2, 1)
            wl[7] = VH(2, 2) + VH(2, 3) + GH(2, 2) + GH(2, 3)
            wl[8] = GH(2, 4) + GH(2, 5) + GH(3, 0)
            wl[9] = GH(3, 1) + VH(3, 0) + VH(3, 1)
            wl[10] = VH(3, 2) + VH(3, 3) + GH(3, 2) + GH(3, 3)
            wl[11] = GH(1, 6) + GH(1, 7)
            wl[12] = GH(2, 6) + GH(2, 7)
            wl[13] = GH(3, 4) + GH(3, 5)
            wl[14] = GH(3, 6) + GH(3, 7)

            woven = set()

            def emit_weave(fn):
                if id(fn) not in woven:
                    woven.add(id(fn))
                    fn()

            def flush_for_unit(pos):
                for pp2 in range(pos):
                    for fn in wl[pp2]:
                        emit_weave(fn)

            state = {"emitted": 0}
            seen_units = set()
            unit_first_item = {}
            for k, (I, p, J) in enumerate(items):
                if J == 0:
                    unit_first_item[upos[(I, p)]] = k

            def emit_qk_item(k):
                I, p, J = items[k]
                pos = upos[(I, p)]
                if pos not in seen_units:
                    seen_units.add(pos)
                    if pos == 4:
                        load_x(2)
                    elif pos == 7:
                        load_x(3)
                    flush_for_unit(pos)
                qk_exp(I, p, J)

            g_qk(0, 0)
            g_qk(0, 1)
            for k in range(len(items)):
                while state["emitted"] < min(k + 1, len(items)):
                    emit_qk_item(state["emitted"])
                    state["emitted"] += 1
                I, p, J = items[k]
                pos = upos[(I, p)]
                if k == 0:
                    for kk in range(1 + LOOKAHEAD):
                        emit_qk_item(state["emitted"])
                        state["emitted"] += 1
                if (I, p) == (0, 0):  # block-0 v group J just before PV J
                    g_v(0, J)
                pv(I, p, J)
                if J == 4 * I + 3:
                    out_stage(I, p, use_act=(pos < 10 or pos == 15),
                              use_pe_bcast=(pos >= 14),
                              muls_on_pool=(pos < 14))
                j_in = k - unit_first_item[pos]
                if j_in < len(wl[pos]):
                    emit_weave(wl[pos][j_in])
                while state["emitted"] < min(k + 2 + LOOKAHEAD, len(items)):
                    emit_qk_item(state["emitted"])
                    state["emitted"] += 1
            for pos in range(len(order)):
                for fn in wl[pos]:
                    emit_weave(fn)
    nc.compile()
    return nc


def _mk_ap(src_ap, offset, dims):
    """Raw strided view of a DRAM tensor (strides/offset in elements)."""
    import concourse.bass as bass

    return bass.AP(tensor=src_ap.tensor, offset=offset, ap=dims)


def _bcast_ap(src_ap, nparts):
    """Partition-broadcast view of a [1, N] DRAM AP -> [nparts, N]."""
    import concourse.bass as bass

    return bass.AP(
        tensor=src_ap.tensor,
        offset=src_ap.offset,
        ap=[[0, nparts]] + list(src_ap.ap)[1:],
    )


def get_nc():
    if "nc" not in _cache:
        _cache["nc"] = _build_nc()
    return _cache["nc"]


def shard_inputs(x, w_attn, b_attn):
    """Full inputs -> per-core input maps (host-side slicing/transposition)."""
    import ml_dtypes

    bf16 = ml_dtypes.bfloat16
    x = np.asarray(x, dtype=np.float32)
    w = np.asarray(w_attn, dtype=np.float32)
    bb = np.asarray(b_attn, dtype=np.float32)
    in_maps = []
    for core in range(N_CORES):
        b, hg = core // 2, core % 2
        r0 = hg * CC  # first q row for this head group
        # pair-interleaved q/k weight columns: [q_p0|k_p0|q_p1|k_p1|...]
        blocks = []
        bcols = []
        for p in range(PAIRS):
            blocks.append(w[r0 + 128 * p : r0 + 128 * (p + 1), :].T)
            blocks.append(w[C + r0 + 128 * p : C + r0 + 128 * (p + 1), :].T)
            bcols.append(bb[r0 + 128 * p : r0 + 128 * (p + 1)])
            bcols.append(bb[C + r0 + 128 * p : C + r0 + 128 * (p + 1)])
        w_qk = np.ascontiguousarray(np.concatenate(blocks, axis=1).astype(bf16))
        b_qk = np.ascontiguousarray(np.stack(bcols, axis=1).astype(np.float32))
        w_v = np.ascontiguousarray(
            w[2 * C + r0 : 2 * C + r0 + CC, :].T.astype(bf16))
        b_v = bb[2 * C + r0 : 2 * C + r0 + CC].reshape(1, CC).astype(np.float32)
        in_maps.append(
            {
                "xT": np.ascontiguousarray(x[b].T.astype(bf16)),
                "w_qk": w_qk,
                "w_v": w_v,
                "b_qk": b_qk,
                "b_v": np.ascontiguousarray(b_v),
                "ones_d": np.ones((1, 128), dtype=np.float32),
            }
        )
    return in_maps


def run(in_maps, trace=False, **kw):
    from concourse import bass_utils

    nc = get_nc()
    return bass_utils.run_bass_kernel_spmd(
        nc, in_maps, core_ids=list(range(N_CORES)), trace=trace, **kw
    )


def gather_output(results):
    y = np.empty((B, T, E), dtype=np.float32)
    for core in range(N_CORES):
        b, hg = core // 2, core % 2
        y[b, :, hg * CC : (hg + 1) * CC] = results[core]["yT"].T
    return y


def kernel(x, w_attn, b_attn):
    in_maps = shard_inputs(x, w_attn, b_attn)
    res = run(in_maps, trace=False)
    return gather_output(res.results)
